# revision 35
# baseline (speedup 1.0000x reference)
"""Trainium2 Bass kernel for nn_Attn_17059610099812.

reference:
    energies = einsum('sh,h->s', encoder_outputs[131072, 512], hidden[512])
    attn = softmax(energies)   -> [1, 1, 131072]

Strategy (8 NeuronCores, SPMD):
  - Shard encoder_outputs along seq_len: 16384 rows per core (host-side split
    via per-core input maps). Host converts inputs to fp16 (free host-side;
    halves HBM traffic — the roofline — and rel err stays ~4e-4 vs the 2e-2
    gate because energies accumulate in fp32).
  - Per core: stream the 16MB fp16 shard through SBUF in 8 x 2MB slabs using
    the "(t p n) h" layout: partition p of slab t holds n=16 consecutive
    rows, i.e. ONE contiguous 16KB descriptor per partition per DMA. Slab
    DMAs alternate between the two HWDGE rings (sync + scalar).
  - Energies split across engines so no single engine is the bottleneck:
    per 128-row group either (a) a fused DVE scalar_tensor_tensor computes
    e[p] = sum_h(E[p, :] * w[:]) with fp32 accumulation, or (b) the
    TensorEngine transposes the group (PE identity matmul), Act copies
    PSUM->SBUF, and a PE matvec against a column-ized hidden accumulates the
    128 energies. pe_num of every pn_rows groups take path (b); the sim-tuned
    split keeps DVE ~43us / Act ~39us / PE ~14us vs DMA ~47us.
  - Local softmax stats per SBUF partition (max via DVE reduce, exp+sum fused
    on ScalarE). Device ships unnormalized exp (fp16) + per-partition
    (max, sum) stats; the host finishes the global softmax normalization (a
    combine over 2KB of stats across cores, negligible) and undoes the
    (p n) permutation.

kernel() accepts the FULL inputs and returns the FULL [1, 1, 131072] output.
"""

import numpy as np

SEQ = 131072
H = 512
NCORES = 8
SHARD = SEQ // NCORES          # 16384 rows per core
NBLK = SHARD // 128            # 128 groups of 128 rows

_CACHE = {}


def slab_plan(pn_rows, pe_num, taper, pe_last=None):
    """Slab sizes (in 128-row groups) + per-slab PE block counts.

    taper: split the first/last big slab into smaller ones so the first
    compute starts earlier and the post-last-byte tail is shorter.
    pe_last: PE share of the final slab (the post-last-byte tail runs on
    PE+Act while the DVE drains its backlog).
    """
    if taper == "head2":
        sizes = [pn_rows // 2] * 2 + [pn_rows] * (NBLK // pn_rows - 1)
    elif taper == "head":
        head = [pn_rows // 4, pn_rows // 4, pn_rows // 2]
        mid_total = NBLK - sum(head)
        assert mid_total % pn_rows == 0
        sizes = head + [pn_rows] * (mid_total // pn_rows)
    elif taper:
        head = [pn_rows // 4, pn_rows // 4, pn_rows // 2]
        tail = [pn_rows // 2, pn_rows // 4, pn_rows // 4]
        mid_total = NBLK - sum(head) - sum(tail)
        assert mid_total % pn_rows == 0
        sizes = head + [pn_rows] * (mid_total // pn_rows) + tail
    else:
        assert NBLK % pn_rows == 0
        sizes = [pn_rows] * (NBLK // pn_rows)
    pes = [min(s, round(s * pe_num / pn_rows)) for s in sizes]
    if pe_last is not None:
        pes[-1] = min(sizes[-1], pe_last)
    return sizes, pes


def _build_program(pn_rows=16, n_queues=3, big_bufs=6, out_split=3, pe_num=0,
                   taper=False, out_q="", pe_last=None, repeat=1):
    import concourse.bacc as bacc
    import concourse.mybir as mybir
    import concourse.tile as tile
    from concourse import masks

    f32 = mybir.dt.float32
    f16 = mybir.dt.float16
    Alu = mybir.AluOpType
    Act = mybir.ActivationFunctionType
    Ax = mybir.AxisListType

    nc = bacc.Bacc(
        "TRN2", target_bir_lowering=False, debug=False, num_devices=NCORES
    )

    enc = nc.dram_tensor("enc", [SHARD, H], f16, kind="ExternalInput")
    hid = nc.dram_tensor("hid", [1, H], f16, kind="ExternalInput")
    attn = nc.dram_tensor("attn", [SHARD], f16, kind="ExternalOutput")
    stats_out = nc.dram_tensor("stats", [2, 128], f32, kind="ExternalOutput")

    sizes, pes = slab_plan(pn_rows, pe_num, taper, pe_last)
    T = len(sizes)

    from contextlib import ExitStack

    with tile.TileContext(nc) as tc, ExitStack() as stack:
        if True:
            small_pool = stack.enter_context(tc.tile_pool(name="small", bufs=1))
            scratch_pool = stack.enter_context(
                tc.tile_pool(name="scratch", bufs=2))
            tsb_pool = stack.enter_context(tc.tile_pool(name="tsb", bufs=4))
            psum_pool = stack.enter_context(
                tc.tile_pool(name="psum", bufs=1, space="PSUM"))
            psum2_pool = stack.enter_context(
                tc.tile_pool(name="psum2", bufs=2, space="PSUM"))
            # one pool per slab size; small head/tail slabs need few buffers.
            # Entered last -> released first (pools must close in LIFO order).
            big_pools = {
                s: stack.enter_context(
                    tc.tile_pool(
                        name=f"big{s}",
                        bufs=min(big_bufs if s == pn_rows else 3,
                                 sizes.count(s)),
                    )
                )
                for s in sorted(set(sizes))
            }
            def dma_eng(i):
                return (nc.sync, nc.scalar, nc.gpsimd)[i % n_queues]

            for _rep in range(repeat):
                # ---- start the big slab loads first (longest pole) ----
                g0s = [sum(sizes[:t]) for t in range(T)]

                def slab_src(t):
                    g0, s = g0s[t], sizes[t]
                    return enc[128 * g0 : 128 * (g0 + s)].rearrange(
                        "(p n) h -> p (n h)", p=128, n=s
                    )

                slabs = []
                for t in range(min(4, T)):
                    slab = big_pools[sizes[t]].tile(
                        [128, sizes[t] * H], f16, tag=f"blk{sizes[t]}"
                    )
                    dma_eng(t).dma_start(slab[:], slab_src(t))
                    slabs.append(slab)

                # ---- setup: broadcast hidden across the 128 partitions ----
                ones_t = small_pool.tile([1, 128], f16, tag="ones")
                nc.vector.memset(ones_t[:], 1.0)
                hid_sb = small_pool.tile([1, H], f16, tag="hid")
                nc.gpsimd.dma_start(hid_sb[:], hid[:])
                w_ps = psum_pool.tile([128, H], f32, tag="wps")
                nc.tensor.matmul(w_ps[:], ones_t[:], hid_sb[:], start=True,
                                 stop=True)
                w_sb = small_pool.tile([128, H], f16, tag="w")
                nc.scalar.copy(w_sb[:], w_ps[:])

                identity = small_pool.tile([128, 128], f32, tag="ident")
                masks.make_identity(nc, identity[:])
                ident16 = small_pool.tile([128, 128], f16, tag="ident16")
                nc.vector.tensor_copy(ident16[:], identity[:])

                if pe_num:
                    # wcol[h, k] = hidden[k*128 + h]: hidden as 4 columns of
                    # 128, for PE matvec against transposed slab chunks.
                    # Built by accumulating hid chunk k against a one-hot row.
                    hots = small_pool.tile([1, 4 * 4], f16, tag="hots")
                    nc.vector.memset(hots[:], 0.0)
                    hots_v = hots[:].rearrange("a (k f) -> a k f", k=4)
                    for k in range(4):
                        nc.vector.memset(hots_v[:, k, k : k + 1], 1.0)
                    wcol_ps = psum_pool.tile([128, 4], f32, tag="wcol")
                    for k in range(4):
                        nc.tensor.matmul(
                            wcol_ps[:],
                            hid_sb[:, 128 * k : 128 * (k + 1)],
                            hots_v[:, k],
                            start=(k == 0),
                            stop=(k == 3),
                        )
                    wcol = small_pool.tile([128, 4], f16, tag="wcolsb")
                    nc.scalar.copy(wcol[:], wcol_ps[:])

                # ---- energies: e_sb[p, c] = energy of shard row
                # 128*n*t + n*p + j  (c = t*n + j) ----
                e_sb = small_pool.tile([128, NBLK], f32, tag="e")
                for t in range(T):
                    s_t, pe_t, g0 = sizes[t], pes[t], g0s[t]
                    if t < len(slabs):
                        slab = slabs[t]
                    else:
                        slab = big_pools[s_t].tile(
                            [128, s_t * H], f16, tag=f"blk{s_t}"
                        )
                        dma_eng(t).dma_start(slab[:], slab_src(t))
                    # first pe_t row-groups of each slab go through the
                    # TensorEngine (PE transpose + matvec, Act copies); the
                    # rest use the fused DVE multiply+reduce. Offloads the
                    # DVE, which is otherwise the critical engine.
                    if pe_t:
                        e_ps = psum_pool.tile([128, pn_rows], f32, tag="eps")
                    for j in range(pe_t):
                        tps = psum2_pool.tile([128, H], f16, tag="tps")
                        for k in range(4):
                            nc.tensor.transpose(
                                tps[:, 128 * k : 128 * (k + 1)],
                                slab[:, j * H + 128 * k : j * H + 128 * (k + 1)],
                                ident16[:],
                            )
                        tsb = tsb_pool.tile([128, H], f16, tag="tsb")
                        nc.scalar.copy(tsb[:], tps[:])
                        for k in range(4):
                            nc.tensor.matmul(
                                e_ps[:, j : j + 1],
                                tsb[:, 128 * k : 128 * (k + 1)],
                                wcol[:, k : k + 1],
                                start=(k == 0),
                                stop=(k == 3),
                            )
                    if pe_t:
                        nc.scalar.copy(
                            e_sb[:, g0 : g0 + pe_t], e_ps[:, :pe_t]
                        )
                    for j in range(pe_t, s_t):
                        scr = scratch_pool.tile([128, H], f16, tag="ttrv")
                        nc.vector.scalar_tensor_tensor(
                            out=scr[:],
                            in0=slab[:, j * H : (j + 1) * H],
                            scalar=1.0,
                            in1=w_sb[:],
                            op0=Alu.mult,
                            op1=Alu.mult,
                            accum_out=e_sb[:, g0 + j : g0 + j + 1],
                        )

                # ---- local softmax stats (per partition) ----
                m1 = small_pool.tile([128, 1], f32, tag="m1")
                nc.vector.reduce_max(m1[:], e_sb[:], axis=Ax.X)
                neg_m1 = small_pool.tile([128, 1], f32, tag="negm1")
                nc.vector.tensor_scalar_mul(neg_m1[:], m1[:], -1.0)
                p_sb = small_pool.tile([128, NBLK], f16, tag="p")
                s1 = small_pool.tile([128, 1], f32, tag="s1")
                nc.scalar.activation(
                    p_sb[:], e_sb[:], Act.Exp, bias=neg_m1[:], scale=1.0,
                    accum_out=s1[:],
                )

                # stats [128, 2] -> PE transpose to [2, 128] so the DRAM write
                # is 2 contiguous 512B descriptors instead of 128 x 8B.
                stats = small_pool.tile([128, 2], f32, tag="stats")
                nc.vector.tensor_copy(stats[:, 0:1], m1[:])
                nc.vector.tensor_copy(stats[:, 1:2], s1[:])
                st_ps = psum_pool.tile([2, 128], f32, tag="statsT")
                nc.tensor.transpose(st_ps[:], stats[:], identity[:])
                st_sb = small_pool.tile([2, 128], f32, tag="statsTsb")
                nc.scalar.copy(st_sb[:], st_ps[:])
                out_engs = (
                    (nc.gpsimd, nc.sync, nc.scalar)
                    if out_q == "gp"
                    else (nc.sync, nc.scalar, nc.gpsimd)
                )
                out_engs[0].dma_start(stats_out[:], st_sb[:])

                # ship p_sb as-is ([p, c] order); host unpermutes + rescales.
                # Split the store across queues (partition ranges).
                attn_v = attn.ap().rearrange("(p c) -> p c", p=128)
                bounds = [0, 43, 86, 128][: out_split + 1]
                if out_split == 1:
                    bounds = [0, 128]
                for i in range(len(bounds) - 1):
                    lo, hi = bounds[i], bounds[i + 1]
                    out_engs[i % len(out_engs)].dma_start(
                        attn_v[lo:hi], p_sb[lo:hi, :]
                    )

    nc.compile()
    return nc


# tuning knobs used by kernel(); see _build_program
BUILD_KW = {"pn_rows": 16, "n_queues": 2, "big_bufs": 8, "out_split": 3,
            "pe_num": 7, "taper": False}


def _get_program():
    key = ("nc", tuple(sorted(BUILD_KW.items())))
    if key not in _CACHE:
        _CACHE[key] = _build_program(**BUILD_KW)
    return _CACHE[key]


def kernel(hidden, encoder_outputs, _trace=False, _trace_kwargs=None):
    from concourse.bass_utils import run_bass_kernel_spmd

    nc = _get_program()
    hid16 = np.ascontiguousarray(
        np.asarray(hidden, dtype=np.float32).reshape(1, H).astype(np.float16)
    )
    enc16 = np.ascontiguousarray(
        np.asarray(encoder_outputs, dtype=np.float32).astype(np.float16)
    )
    assert enc16.shape == (SEQ, H)

    in_maps = [
        {"enc": enc16[c * SHARD : (c + 1) * SHARD], "hid": hid16}
        for c in range(NCORES)
    ]
    res = run_bass_kernel_spmd(
        nc,
        in_maps,
        core_ids=list(range(NCORES)),
        trace=_trace,
        **(_trace_kwargs or {}),
    )
    _CACHE["last_results"] = res

    u = np.stack([res.results[c]["attn"] for c in range(NCORES)]).astype(
        np.float32).reshape(NCORES, 128, NBLK)      # [core, p, c]
    st = np.stack([res.results[c]["stats"] for c in range(NCORES)])  # [8, 2, 128]
    m = st[:, 0, :].astype(np.float64)   # [8, 128]
    s = st[:, 1, :].astype(np.float64)   # [8, 128]
    g = m.max()
    S = (np.exp(m - g) * s).sum()
    factor = (np.exp(m - g) / S).astype(np.float32)                  # [8, 128]
    u = u * factor[:, :, None]          # [8, 128, NBLK] scaled
    # device ships [p, c] with c = g0(t) + j; shard row = 128*g0 + p*s_t + j
    sizes, _ = slab_plan(BUILD_KW["pn_rows"], BUILD_KW.get("pe_num", 0),
                         BUILD_KW.get("taper", False),
                         BUILD_KW.get("pe_last"))
    out = np.empty((NCORES, SHARD), dtype=np.float32)
    g0 = 0
    for s in sizes:
        blk = u[:, :, g0 : g0 + s]                  # [8, 128, s]
        out[:, 128 * g0 : 128 * (g0 + s)] = blk.reshape(NCORES, 128 * s)
        g0 += s
    return out.reshape(1, 1, SEQ).astype(np.float32)
